# revision 5
# baseline (speedup 1.0000x reference)
"""Trainium2 Bass kernel for a 2-layer GCN with root-node readout.

The reference computes a full-graph 2-layer GCN but only returns h2[roots]
(one root per graph).  Exact algebraic pruning: out[g] depends only on edges
into root g (layer 2) and edges into those edges' sources (layer 1):

  out[g]  = sum_{e2: dst=root_g} norm_e2 * h2[src_e2] + b2
  h2      = relu( (sum_{e1: dst=s} norm_e1 * x[src_e1]) @ W1 + b1 ) @ W2

Sharding: unique roots are split across 8 cores, balanced by layer-1 edge
count.  The host computes norms, roots, per-core edge lists and layouts;
each core streams its layer-1 messages (norm*x rows in bf16; the 2e-2
harness tolerance leaves ~8x margin over the ~2.5e-3 bf16 error) as
128-edge blocks grouped into 64-destination windows.  Scatter-add is a
one-hot matmul per block into a per-window PSUM tile; the one-hot selection
matrices are built on-device with a DVE is_equal against an iota row
(overlapped with the msg DMA stream).  Layer 2 is folded into a small dense
matrix A2 [roots x sources] built on the host from edge norms.
"""

import numpy as np
import ml_dtypes

import concourse.bacc as bacc
import concourse.bass as bass  # noqa: F401
import concourse.mybir as mybir
import concourse.tile as tile
from concourse import bass_utils
from concourse._compat import axon_active


def _ensure_ntff_hook():
    """bass_utils' trace path imports antenv.axon_hooks, which this image
    lacks; synthesize it from trn_agent_boot's ctypes recipe so BASS_TRACE=1
    profiling works. Silent no-op when anything is missing."""
    import sys as _sys
    try:
        import antenv.axon_hooks  # noqa: F401
        return
    except ImportError:
        pass
    try:
        import types as _types
        from trn_agent_boot.trn_boot import _ntff_profile_via_ctypes
        _hook = _ntff_profile_via_ctypes("/opt/axon/libaxon_pjrt.so")
        mod = _types.ModuleType("antenv.axon_hooks")
        mod.get_axon_ntff_profile_hook = lambda: _hook
        mod.set_axon_ntff_profile_hook = lambda h: None
        _sys.modules["antenv.axon_hooks"] = mod
        import antenv as _antenv
        _antenv.axon_hooks = mod
    except Exception:
        pass

N_CORES = 8
P = 128
W64 = 64
HID = 128
OUT_C = 64
R_PAD = 64

F32 = mybir.dt.float32
BF16 = mybir.dt.bfloat16


# ----------------------------------------------------------------------------
# Host-side preprocessing
# ----------------------------------------------------------------------------

def _compute_norm_and_roots(x, edge_index, batch, num_graphs):
    """Replicate reference._gcn_norm and the root-finding logic exactly."""
    n = x.shape[0]
    G = int(num_graphs)
    loop = np.arange(n, dtype=np.int64)
    src = np.concatenate([edge_index[0], loop])
    dst = np.concatenate([edge_index[1], loop])
    deg = np.bincount(dst, minlength=n).astype(np.float64)
    dinv = np.zeros(n, dtype=np.float32)
    nz = deg > 0
    dinv[nz] = (1.0 / np.sqrt(deg[nz])).astype(np.float32)
    norm = (dinv[src] * dinv[dst]).astype(np.float32)

    node_types = x[:, 0]
    idx = np.arange(n, dtype=np.int64)
    cand = np.where(node_types == 0.0, idx, n)
    roots = np.full(G, np.iinfo(np.int64).max, dtype=np.int64)
    bc = np.clip(batch, 0, G - 1)
    np.minimum.at(roots, bc, cand)
    valid = np.zeros(G, dtype=bool)
    valid[bc] = True
    roots[~valid] = np.iinfo(np.int64).max
    roots = np.clip(roots, 0, n - 1)  # jax out-of-bounds gather clamps
    return src, dst, norm, roots, deg.astype(np.int64)


def _build_shards(x, edge_index, batch, num_graphs, W1, W2, b1, b2):
    n = x.shape[0]
    src, dst, norm, roots, deg = _compute_norm_and_roots(
        x, edge_index, batch, num_graphs)

    uroots, inv_map = np.unique(roots, return_inverse=True)
    U = len(uroots)

    order = np.argsort(dst, kind="stable")
    dst_s = dst[order]
    src_s = src[order]
    norm_s = norm[order]
    starts = np.searchsorted(dst_s, np.arange(n))
    ends = np.searchsorted(dst_s, np.arange(n) + 1)

    # balance roots across cores by estimated layer-1 edge load
    root_load = np.array(
        [deg[src_s[starts[r]:ends[r]]].sum() for r in uroots], dtype=np.int64)
    core_of_root = np.zeros(U, dtype=np.int64)
    core_load = np.zeros(N_CORES, dtype=np.int64)
    core_cnt = np.zeros(N_CORES, dtype=np.int64)
    for ri in np.argsort(-root_load):
        ok = core_cnt < R_PAD
        c = np.flatnonzero(ok)[np.argmin(core_load[ok])]
        core_of_root[ri] = c
        core_load[c] += root_load[ri]
        core_cnt[c] += 1

    cores = []
    for c in range(N_CORES):
        R_c = uroots[core_of_root == c]
        if len(R_c):
            e2_idx = np.concatenate(
                [np.arange(starts[r], ends[r]) for r in R_c])
        else:
            e2_idx = np.array([], dtype=np.int64)
        e2_src = src_s[e2_idx]
        e2_dst = dst_s[e2_idx]
        e2_norm = norm_s[e2_idx]
        S = np.unique(e2_src)
        cores.append(dict(R_c=R_c, e2_src=e2_src, e2_dst=e2_dst,
                          e2_norm=e2_norm, S=S))

    nS_max = max(max(len(c["S"]) for c in cores), 1)
    nW2 = -(-nS_max // P)        # 128-wide pair chunks (phase 2 granularity)
    nW = 2 * nW2                 # 64-wide scatter windows

    # per-core window assignment: bin-pack S nodes into nW windows (<=64
    # nodes each) balancing per-window layer-1 edge counts
    for c in cores:
        S = c["S"]
        nS = len(S)
        w_nodes = np.zeros(nW, dtype=np.int64)
        w_edges = np.zeros(nW, dtype=np.int64)
        s_window = np.zeros(max(nS, 1), dtype=np.int64)
        s_slot = np.zeros(max(nS, 1), dtype=np.int64)
        degS = deg[S] if nS else np.zeros(0, dtype=np.int64)
        for si in np.argsort(-degS, kind="stable"):
            ok = w_nodes < W64
            w = np.flatnonzero(ok)[np.argmin(w_edges[ok])]
            s_window[si] = w
            s_slot[si] = w_nodes[w]
            w_nodes[w] += 1
            w_edges[w] += degS[si]
        c["s_pos"] = s_window * W64 + s_slot  # position of S[i] in [0, nW*64)
        c["w_edges"] = w_edges

        R_c = c["R_c"]
        A2 = np.zeros((R_PAD, nW * W64), dtype=np.float32)
        if nS:
            r_pos = np.searchsorted(R_c, c["e2_dst"])
            s_pos2 = c["s_pos"][np.searchsorted(S, c["e2_src"])]
            np.add.at(A2, (r_pos, s_pos2), c["e2_norm"])
        c["A2"] = A2

    B = np.zeros(nW, dtype=np.int64)
    for c in cores:
        B = np.maximum(B, -(-c["w_edges"] // P))
    B = np.maximum(B, 1)
    n_blk = int(B.sum())
    blk0 = np.concatenate([[0], np.cumsum(B)])  # first block of each window

    per_core = []
    for c in cores:
        S = c["S"]
        nS = len(S)
        msg = np.zeros((n_blk * P, HID), dtype=np.float32)
        dstrel = np.zeros(n_blk * P, dtype=np.float32)
        if nS:
            e1_idx = np.concatenate(
                [np.arange(starts[s], ends[s]) for s in S])
            e1_src = src_s[e1_idx]
            e1_pos = c["s_pos"][np.searchsorted(S, dst_s[e1_idx])]
            e1_norm = norm_s[e1_idx]
            o = np.argsort(e1_pos // W64, kind="stable")
            e1_src, e1_pos, e1_norm = e1_src[o], e1_pos[o], e1_norm[o]
            w_of_e = e1_pos // W64
            pos = 0
            for w in range(nW):
                sel = w_of_e == w
                k = int(sel.sum())
                if k:
                    base = blk0[w] * P
                    msg[base:base + k] = e1_norm[sel, None] * x[e1_src[sel]]
                    dstrel[base:base + k] = (e1_pos[sel] - w * W64).astype(
                        np.float32)
        m2 = np.ascontiguousarray(
            msg.astype(ml_dtypes.bfloat16).reshape(n_blk, P, HID)
            .transpose(1, 0, 2))
        dr = np.ascontiguousarray(
            dstrel.reshape(n_blk, P).T.astype(ml_dtypes.bfloat16))
        A2T = np.zeros((P, nW2, R_PAD), dtype=ml_dtypes.bfloat16)
        A2T[:, :, :] = (c["A2"].T.reshape(nW2, P, R_PAD)
                        .transpose(1, 0, 2).astype(ml_dtypes.bfloat16))
        per_core.append(dict(msg=m2, dr=dr, A2T=A2T, R_c=c["R_c"]))

    # cbA (bf16, ships first on the fast sync queue):
    #   [dstrel | iota64 | W1 | A2T | W2]
    iota = np.tile(np.arange(W64, dtype=np.float32), (P, 1)).astype(
        ml_dtypes.bfloat16)
    W1b = np.ascontiguousarray(W1.astype(ml_dtypes.bfloat16))
    W2b = np.ascontiguousarray(W2.astype(ml_dtypes.bfloat16))
    # cf32: [b1 | b2pad]
    b2pad = np.zeros((P, OUT_C), dtype=np.float32)
    b2pad[:R_PAD] = np.tile(b2.astype(np.float32), (R_PAD, 1))
    cf32 = np.ascontiguousarray(np.concatenate(
        [b1.astype(np.float32).reshape(HID, 1), b2pad], axis=1))
    for pc in per_core:
        pc["cbA"] = np.ascontiguousarray(np.concatenate(
            [pc.pop("dr"), iota, W1b,
             pc.pop("A2T").reshape(P, nW2 * R_PAD), W2b], axis=1))
        pc["cf32"] = cf32
    meta = dict(nW=nW, nW2=nW2, B=[int(v) for v in B], n_blk=n_blk, U=U,
                inv_map=inv_map,
                root_of=[pc.pop("R_c") for pc in per_core])
    return per_core, meta


# ----------------------------------------------------------------------------
# Device program
# ----------------------------------------------------------------------------

def _build_program(nW, nW2, B, n_blk):
    nc = bacc.Bacc("TRN2", target_bir_lowering=False, debug=not axon_active(),
                   num_devices=N_CORES)
    msg_d = nc.dram_tensor("msg", [P, n_blk, HID], BF16,
                           kind="ExternalInput").ap()
    cba_w = n_blk + W64 + HID + nW2 * R_PAD + OUT_C
    cba_d = nc.dram_tensor("cbA", [P, cba_w], BF16, kind="ExternalInput").ap()
    cf32_d = nc.dram_tensor("cf32", [P, 1 + OUT_C], F32,
                            kind="ExternalInput").ap()
    out_d = nc.dram_tensor("out", [R_PAD, OUT_C], F32, kind="ExternalOutput").ap()

    blk0 = [0]
    for w in range(nW):
        blk0.append(blk0[-1] + int(B[w]))

    with tile.TileContext(nc) as tc:
        with (
            tc.tile_pool(name="const", bufs=1) as const,
            tc.tile_pool(name="small", bufs=3) as small,
            tc.tile_pool(name="psagg", bufs=3, space="PSUM") as psagg,
            tc.tile_pool(name="ps1", bufs=2, space="PSUM") as ps1,
            tc.tile_pool(name="psout", bufs=1, space="PSUM") as psout,
        ):
            # the S-gen-critical blob goes FIRST on the fast sync queue
            cba = const.tile([P, cba_w], BF16, tag="cbA")
            nc.sync.dma_start(cba[:], cba_d)
            dr_sb = cba[:, 0:n_blk]
            o = n_blk
            iota_sb = cba[:, o:o + W64]; o += W64
            w1_sb = cba[:, o:o + HID]; o += HID
            a2_sb = cba[:, o:o + nW2 * R_PAD]; o += nW2 * R_PAD
            w2_sb = cba[:, o:o + OUT_C]
            cf32 = const.tile([P, 1 + OUT_C], F32, tag="cf32")
            nc.sync.dma_start(cf32[:], cf32_d)
            b1_sb = cf32[:, 0:1]
            b2_sb = cf32[:R_PAD, 1:1 + OUT_C]

            # msg DMA in window-pair chunks (~0.6 MB each) for DMA efficiency
            msg_p = []
            for p in range(nW2):
                b0, b1e = blk0[2 * p], blk0[2 * p + 2]
                mt = const.tile([P, b1e - b0, HID], BF16, tag=f"msg{p}",
                                name=f"msg{p}")
                nc.sync.dma_start(mt[:], msg_d[:, b0:b1e, :])
                msg_p.append(mt)

            # S generation: one-hot(dstrel), one batched DVE op per pair
            s_p = []
            for p in range(nW2):
                b0, b1e = blk0[2 * p], blk0[2 * p + 2]
                st = const.tile([P, b1e - b0, W64], BF16, tag=f"S{p}",
                                name=f"S{p}")
                nc.vector.tensor_tensor(
                    out=st[:],
                    in0=dr_sb[:, b0:b1e, None].to_broadcast(
                        [P, b1e - b0, W64]),
                    in1=iota_sb[:, None, :].to_broadcast([P, b1e - b0, W64]),
                    op=mybir.AluOpType.is_equal)
                s_p.append(st)

            out_ps = psout.tile([R_PAD, OUT_C], F32, tag="outps")

            # Phase-2 (W1/relu/W2/A2) is a 4-deep software pipeline: its
            # stages for pairs q-1/q-2/q-3 are emitted between pair q's
            # scatter blocks so every cross-engine dependency has a full
            # pair-step (~1.9us) of slack and the in-order PE queue never
            # stalls on an ACT round-trip.
            pre = []
            agg1_t = {}
            relu_t = {}
            h2ps_t = {}
            h2sb_t = {}

            def stage_w1(q):
                p_agg1 = ps1.tile([HID, P], F32, tag="agg1", name=f"agg1_{q}")
                nc.tensor.matmul(out=p_agg1[:], lhsT=w1_sb, rhs=pre[q][:],
                                 start=True, stop=True)
                agg1_t[q] = p_agg1
                relu_w = small.tile([HID, P], BF16, tag="relu",
                                    name=f"relu_{q}")
                nc.scalar.activation(out=relu_w[:], in_=p_agg1[:],
                                     func=mybir.ActivationFunctionType.Relu,
                                     bias=b1_sb, scale=1.0)
                relu_t[q] = relu_w

            def stage_w2(q):
                p_h2 = ps1.tile([P, OUT_C], F32, tag="h2", name=f"h2_{q}")
                nc.tensor.matmul(out=p_h2[:], lhsT=relu_t[q][:], rhs=w2_sb,
                                 start=True, stop=True)
                h2ps_t[q] = p_h2
                h2_sb = small.tile([P, OUT_C], BF16, tag="h2sb",
                                   name=f"h2sb_{q}")
                nc.scalar.copy(out=h2_sb[:], in_=p_h2[:])
                h2sb_t[q] = h2_sb

            def stage_a2(q):
                nc.tensor.matmul(out=out_ps[:],
                                 lhsT=a2_sb[:, q * R_PAD:(q + 1) * R_PAD],
                                 rhs=h2sb_t.pop(q)[:],
                                 start=(q == 0), stop=(q == nW2 - 1))

            for q in range(nW2 + 3):
                if q < nW2:
                    mt = msg_p[q]
                    pw = psagg.tile([P, P], F32, tag="pw", name=f"pw{q}")
                    for h in range(2):
                        w = 2 * q + h
                        Bw = int(B[w])
                        mb0 = blk0[w] - blk0[2 * q]
                        cols = slice(h * W64, (h + 1) * W64)
                        for b in range(Bw):
                            nc.tensor.matmul(out=pw[:, cols],
                                             lhsT=mt[:, mb0 + b, :],
                                             rhs=s_p[q][:, mb0 + b, :],
                                             start=(b == 0),
                                             stop=(b == Bw - 1))
                    pre.append(const.tile([P, P], BF16, tag=f"pre{q}",
                                          name=f"pre{q}"))
                    nc.scalar.copy(out=pre[q][:], in_=pw[:])
                if 0 <= q - 1 < nW2:
                    stage_w1(q - 1)
                if 0 <= q - 2 < nW2:
                    stage_w2(q - 2)
                if 0 <= q - 3 < nW2:
                    stage_a2(q - 3)

            out_sb = const.tile([R_PAD, OUT_C], F32, tag="outsb")
            nc.vector.tensor_add(out=out_sb[:], in0=out_ps[:], in1=b2_sb)
            nc.sync.dma_start(out_d, out_sb[:])

    nc.compile()
    return nc


# ----------------------------------------------------------------------------
# Entry point
# ----------------------------------------------------------------------------

_RESULT_CACHE = {}


def kernel(x, edge_index, batch, num_graphs, W1, b1, W2, b2, **_ignored):
    x = np.ascontiguousarray(np.asarray(x, dtype=np.float32))
    edge_index = np.asarray(edge_index).astype(np.int64)
    batch = np.asarray(batch).astype(np.int64)
    G = int(np.asarray(num_graphs))
    W1 = np.asarray(W1, dtype=np.float32)
    b1 = np.asarray(b1, dtype=np.float32)
    W2 = np.asarray(W2, dtype=np.float32)
    b2 = np.asarray(b2, dtype=np.float32)

    per_core, meta = _build_shards(x, edge_index, batch, G, W1, W2, b1, b2)
    nc = _build_program(meta["nW"], meta["nW2"], meta["B"], meta["n_blk"])

    in_maps = [dict(pc) for pc in per_core]

    _ensure_ntff_hook()
    try:
        res = bass_utils.run_bass_kernel_spmd(nc, in_maps,
                                              core_ids=list(range(N_CORES)))
    except Exception:
        # transient device wedge (NRT_EXEC_UNIT_UNRECOVERABLE) or profiling
        # hiccup: retry once with tracing off and a core reset requested
        import os as _os
        _os.environ["BASS_NEVER_TRACE"] = "1"
        _os.environ.setdefault("NEURON_RT_RESET_CORES", "1")
        res = bass_utils.run_bass_kernel_spmd(nc, in_maps,
                                              core_ids=list(range(N_CORES)))
    # reassemble: core c's output row i corresponds to unique root
    # meta["root_of"][c][i]; uroots (sorted) is indexed by inv_map
    out_u = np.zeros((meta["U"], OUT_C), dtype=np.float32)
    uroots = np.sort(np.concatenate(meta["root_of"]))
    pos_of = {int(r): j for j, r in enumerate(uroots)}
    for c in range(N_CORES):
        o = np.asarray(res.results[c]["out"])
        for i, r in enumerate(meta["root_of"][c]):
            out_u[pos_of[int(r)]] = o[i]
    out = out_u[meta["inv_map"]].astype(np.float32)
    # kernel() may be probed; stash the bass results for test harness use
    _RESULT_CACHE["last"] = res
    return out


# revision 6
# speedup vs baseline: 1.1336x; 1.1336x over previous
"""Trainium2 Bass kernel for a 2-layer GCN with root-node readout.

The reference computes a full-graph 2-layer GCN but only returns h2[roots]
(one root per graph).  Exact algebraic pruning: out[g] depends only on edges
into root g (layer 2) and edges into those edges' sources (layer 1):

  out[g]  = sum_{e2: dst=root_g} norm_e2 * h2[src_e2] + b2
  h2      = relu( (sum_{e1: dst=s} norm_e1 * x[src_e1]) @ W1 + b1 ) @ W2

Sharding: unique roots are split across 8 cores, balanced by layer-1 edge
count.  Each core streams its layer-1 messages (norm*x rows) as 128-edge
blocks grouped into 64-destination windows.  Messages AND their one-hot
destination-selection matrices ship as fp8e4 (the 2e-2 harness tolerance
leaves ~2x margin over the ~1.1e-2 fp8 error), interleaved per block so one
DMA per window-pair feeds both matmul operands.  Scatter-add is a one-hot
matmul per block into a per-pair PSUM tile; fp8 weights get 4x fast-weight-
load so the PE tracks the DMA stream.  Layer 2 is folded into a small dense
matrix A2 [roots x sources] built on the host from edge norms; the
W1/relu/W2/A2 chain runs as a 4-deep software pipeline interleaved with the
scatter stream.
"""

import numpy as np
import ml_dtypes

import concourse.bacc as bacc
import concourse.bass as bass  # noqa: F401
import concourse.mybir as mybir
import concourse.tile as tile
from concourse import bass_utils
from concourse._compat import axon_active


def _ensure_ntff_hook():
    """bass_utils' trace path imports antenv.axon_hooks, which this image
    lacks; synthesize it from trn_agent_boot's ctypes recipe so BASS_TRACE=1
    profiling works. Silent no-op when anything is missing."""
    import sys as _sys
    try:
        import antenv.axon_hooks  # noqa: F401
        return
    except ImportError:
        pass
    try:
        import types as _types
        from trn_agent_boot.trn_boot import _ntff_profile_via_ctypes
        _hook = _ntff_profile_via_ctypes("/opt/axon/libaxon_pjrt.so")
        mod = _types.ModuleType("antenv.axon_hooks")
        mod.get_axon_ntff_profile_hook = lambda: _hook
        mod.set_axon_ntff_profile_hook = lambda h: None
        _sys.modules["antenv.axon_hooks"] = mod
        import antenv as _antenv
        _antenv.axon_hooks = mod
    except Exception:
        pass

N_CORES = 8
P = 128
W64 = 64
HID = 128
OUT_C = 64
R_PAD = 64
BLK_W = HID + W64          # per-block row: [msg 128 | one-hot 64] fp8

F32 = mybir.dt.float32
BF16 = mybir.dt.bfloat16
FP8 = mybir.dt.float8e4


# ----------------------------------------------------------------------------
# Host-side preprocessing
# ----------------------------------------------------------------------------

def _compute_norm_and_roots(x, edge_index, batch, num_graphs):
    """Replicate reference._gcn_norm and the root-finding logic exactly."""
    n = x.shape[0]
    G = int(num_graphs)
    loop = np.arange(n, dtype=np.int64)
    src = np.concatenate([edge_index[0], loop])
    dst = np.concatenate([edge_index[1], loop])
    deg = np.bincount(dst, minlength=n).astype(np.float64)
    dinv = np.zeros(n, dtype=np.float32)
    nz = deg > 0
    dinv[nz] = (1.0 / np.sqrt(deg[nz])).astype(np.float32)
    norm = (dinv[src] * dinv[dst]).astype(np.float32)

    node_types = x[:, 0]
    idx = np.arange(n, dtype=np.int64)
    cand = np.where(node_types == 0.0, idx, n)
    roots = np.full(G, np.iinfo(np.int64).max, dtype=np.int64)
    bc = np.clip(batch, 0, G - 1)
    np.minimum.at(roots, bc, cand)
    valid = np.zeros(G, dtype=bool)
    valid[bc] = True
    roots[~valid] = np.iinfo(np.int64).max
    roots = np.clip(roots, 0, n - 1)  # jax out-of-bounds gather clamps
    return src, dst, norm, roots, deg.astype(np.int64)


def _build_shards(x, edge_index, batch, num_graphs, W1, W2, b1, b2):
    n = x.shape[0]
    src, dst, norm, roots, deg = _compute_norm_and_roots(
        x, edge_index, batch, num_graphs)

    uroots, inv_map = np.unique(roots, return_inverse=True)
    U = len(uroots)

    order = np.argsort(dst, kind="stable")
    dst_s = dst[order]
    src_s = src[order]
    norm_s = norm[order]
    starts = np.searchsorted(dst_s, np.arange(n))
    ends = np.searchsorted(dst_s, np.arange(n) + 1)

    # balance roots across cores by estimated layer-1 edge load
    root_load = np.array(
        [deg[src_s[starts[r]:ends[r]]].sum() for r in uroots], dtype=np.int64)
    core_of_root = np.zeros(U, dtype=np.int64)
    core_load = np.zeros(N_CORES, dtype=np.int64)
    core_cnt = np.zeros(N_CORES, dtype=np.int64)
    for ri in np.argsort(-root_load):
        ok = core_cnt < R_PAD
        c = np.flatnonzero(ok)[np.argmin(core_load[ok])]
        core_of_root[ri] = c
        core_load[c] += root_load[ri]
        core_cnt[c] += 1

    cores = []
    for c in range(N_CORES):
        R_c = uroots[core_of_root == c]
        if len(R_c):
            e2_idx = np.concatenate(
                [np.arange(starts[r], ends[r]) for r in R_c])
        else:
            e2_idx = np.array([], dtype=np.int64)
        e2_src = src_s[e2_idx]
        e2_dst = dst_s[e2_idx]
        e2_norm = norm_s[e2_idx]
        S = np.unique(e2_src)
        cores.append(dict(R_c=R_c, e2_src=e2_src, e2_dst=e2_dst,
                          e2_norm=e2_norm, S=S))

    nS_max = max(max(len(c["S"]) for c in cores), 1)
    nW2 = -(-nS_max // P)        # 128-wide pair chunks (phase 2 granularity)
    nW = 2 * nW2                 # 64-wide scatter windows

    # per-core window assignment: bin-pack S nodes into nW windows (<=64
    # nodes each) balancing per-window layer-1 edge counts
    for c in cores:
        S = c["S"]
        nS = len(S)
        w_nodes = np.zeros(nW, dtype=np.int64)
        w_edges = np.zeros(nW, dtype=np.int64)
        s_window = np.zeros(max(nS, 1), dtype=np.int64)
        s_slot = np.zeros(max(nS, 1), dtype=np.int64)
        degS = deg[S] if nS else np.zeros(0, dtype=np.int64)
        for si in np.argsort(-degS, kind="stable"):
            ok = w_nodes < W64
            w = np.flatnonzero(ok)[np.argmin(w_edges[ok])]
            s_window[si] = w
            s_slot[si] = w_nodes[w]
            w_nodes[w] += 1
            w_edges[w] += degS[si]
        c["s_pos"] = s_window * W64 + s_slot  # position of S[i] in [0, nW*64)
        c["w_edges"] = w_edges

        R_c = c["R_c"]
        A2 = np.zeros((R_PAD, nW * W64), dtype=np.float32)
        if nS:
            r_pos = np.searchsorted(R_c, c["e2_dst"])
            s_pos2 = c["s_pos"][np.searchsorted(S, c["e2_src"])]
            np.add.at(A2, (r_pos, s_pos2), c["e2_norm"])
        c["A2"] = A2

    B = np.zeros(nW, dtype=np.int64)
    for c in cores:
        B = np.maximum(B, -(-c["w_edges"] // P))
    B = np.maximum(B, 1)
    n_blk = int(B.sum())
    blk0 = np.concatenate([[0], np.cumsum(B)])  # first block of each window

    per_core = []
    for c in cores:
        S = c["S"]
        nS = len(S)
        msg = np.zeros((n_blk * P, HID), dtype=np.float32)
        onehot = np.zeros((n_blk * P, W64), dtype=ml_dtypes.float8_e4m3)
        if nS:
            e1_idx = np.concatenate(
                [np.arange(starts[s], ends[s]) for s in S])
            e1_src = src_s[e1_idx]
            e1_pos = c["s_pos"][np.searchsorted(S, dst_s[e1_idx])]
            e1_norm = norm_s[e1_idx]
            o = np.argsort(e1_pos // W64, kind="stable")
            e1_src, e1_pos, e1_norm = e1_src[o], e1_pos[o], e1_norm[o]
            w_of_e = e1_pos // W64
            for w in range(nW):
                sel = w_of_e == w
                k = int(sel.sum())
                if k:
                    base = blk0[w] * P
                    msg[base:base + k] = e1_norm[sel, None] * x[e1_src[sel]]
                    onehot[base + np.arange(k), e1_pos[sel] - w * W64] = 1.0
        ms = np.empty((P, n_blk, BLK_W), dtype=ml_dtypes.float8_e4m3)
        ms[:, :, :HID] = (msg.astype(ml_dtypes.float8_e4m3)
                          .reshape(n_blk, P, HID).transpose(1, 0, 2))
        ms[:, :, HID:] = onehot.reshape(n_blk, P, W64).transpose(1, 0, 2)
        A2T = (c["A2"].T.reshape(nW2, P, R_PAD)
               .transpose(1, 0, 2).astype(ml_dtypes.bfloat16))
        per_core.append(dict(ms=np.ascontiguousarray(ms), A2T=A2T,
                             R_c=c["R_c"]))

    # cbA (bf16): [W1 | A2T | W2]; cf32: [b1 | b2pad]
    W1b = W1.astype(ml_dtypes.bfloat16)
    W2b = W2.astype(ml_dtypes.bfloat16)
    b2pad = np.zeros((P, OUT_C), dtype=np.float32)
    b2pad[:R_PAD] = np.tile(b2.astype(np.float32), (R_PAD, 1))
    cf32 = np.ascontiguousarray(np.concatenate(
        [b1.astype(np.float32).reshape(HID, 1), b2pad], axis=1))
    for pc in per_core:
        pc["cbA"] = np.ascontiguousarray(np.concatenate(
            [W1b, pc.pop("A2T").reshape(P, nW2 * R_PAD), W2b], axis=1))
        pc["cf32"] = cf32
    meta = dict(nW=nW, nW2=nW2, B=[int(v) for v in B], n_blk=n_blk, U=U,
                inv_map=inv_map,
                root_of=[pc.pop("R_c") for pc in per_core])
    return per_core, meta


# ----------------------------------------------------------------------------
# Device program
# ----------------------------------------------------------------------------

def _build_program(nW, nW2, B, n_blk):
    nc = bacc.Bacc("TRN2", target_bir_lowering=False, debug=not axon_active(),
                   num_devices=N_CORES)
    ms_d = nc.dram_tensor("ms", [P, n_blk, BLK_W], FP8,
                          kind="ExternalInput").ap()
    cba_w = HID + nW2 * R_PAD + OUT_C
    cba_d = nc.dram_tensor("cbA", [P, cba_w], BF16, kind="ExternalInput").ap()
    cf32_d = nc.dram_tensor("cf32", [P, 1 + OUT_C], F32,
                            kind="ExternalInput").ap()
    out_d = nc.dram_tensor("out", [R_PAD, OUT_C], F32, kind="ExternalOutput").ap()

    blk0 = [0]
    for w in range(nW):
        blk0.append(blk0[-1] + int(B[w]))

    with tile.TileContext(nc) as tc:
        with (
            tc.tile_pool(name="const", bufs=1) as const,
            tc.tile_pool(name="small", bufs=3) as small,
            tc.tile_pool(name="psagg", bufs=3, space="PSUM") as psagg,
            tc.tile_pool(name="ps1", bufs=2, space="PSUM") as ps1,
            tc.tile_pool(name="psout", bufs=1, space="PSUM") as psout,
        ):
            cba = const.tile([P, cba_w], BF16, tag="cbA")
            nc.sync.dma_start(cba[:], cba_d)
            w1_sb = cba[:, 0:HID]
            a2_sb = cba[:, HID:HID + nW2 * R_PAD]
            w2_sb = cba[:, HID + nW2 * R_PAD:]
            cf32 = const.tile([P, 1 + OUT_C], F32, tag="cf32")
            nc.sync.dma_start(cf32[:], cf32_d)
            b1_sb = cf32[:, 0:1]
            b2_sb = cf32[:R_PAD, 1:1 + OUT_C]

            # msg+onehot DMA in window-pair chunks (~0.44 MB each)
            ms_p = []
            for p in range(nW2):
                b0, b1e = blk0[2 * p], blk0[2 * p + 2]
                mt = const.tile([P, b1e - b0, BLK_W], FP8, tag=f"ms{p}",
                                name=f"ms{p}")
                nc.sync.dma_start(mt[:], ms_d[:, b0:b1e, :])
                ms_p.append(mt)

            out_ps = psout.tile([R_PAD, OUT_C], F32, tag="outps")

            # Phase-2 (W1/relu/W2/A2) is a 4-deep software pipeline: its
            # stages for pairs q-1/q-2/q-3 are emitted between pair q's
            # scatter blocks so every cross-engine dependency has a full
            # pair-step of slack and the in-order PE queue never stalls.
            pre = []
            relu_t = {}
            h2sb_t = {}

            def stage_w1(q):
                p_agg1 = ps1.tile([HID, P], F32, tag="agg1", name=f"agg1_{q}")
                nc.tensor.matmul(out=p_agg1[:], lhsT=w1_sb, rhs=pre[q][:],
                                 start=True, stop=True)
                relu_w = small.tile([HID, P], BF16, tag="relu",
                                    name=f"relu_{q}")
                nc.scalar.activation(out=relu_w[:], in_=p_agg1[:],
                                     func=mybir.ActivationFunctionType.Relu,
                                     bias=b1_sb, scale=1.0)
                relu_t[q] = relu_w

            def stage_w2(q):
                p_h2 = ps1.tile([P, OUT_C], F32, tag="h2", name=f"h2_{q}")
                nc.tensor.matmul(out=p_h2[:], lhsT=relu_t.pop(q)[:],
                                 rhs=w2_sb, start=True, stop=True)
                h2_sb = small.tile([P, OUT_C], BF16, tag="h2sb",
                                   name=f"h2sb_{q}")
                nc.scalar.copy(out=h2_sb[:], in_=p_h2[:])
                h2sb_t[q] = h2_sb

            def stage_a2(q):
                nc.tensor.matmul(out=out_ps[:],
                                 lhsT=a2_sb[:, q * R_PAD:(q + 1) * R_PAD],
                                 rhs=h2sb_t.pop(q)[:],
                                 start=(q == 0), stop=(q == nW2 - 1))

            for q in range(nW2 + 3):
                if q < nW2:
                    mt = ms_p[q]
                    pw = psagg.tile([P, P], F32, tag="pw", name=f"pw{q}")
                    for h in range(2):
                        w = 2 * q + h
                        Bw = int(B[w])
                        mb0 = blk0[w] - blk0[2 * q]
                        cols = slice(h * W64, (h + 1) * W64)
                        for b in range(Bw):
                            nc.tensor.matmul(out=pw[:, cols],
                                             lhsT=mt[:, mb0 + b, 0:HID],
                                             rhs=mt[:, mb0 + b, HID:BLK_W],
                                             start=(b == 0),
                                             stop=(b == Bw - 1))
                    pre.append(const.tile([P, P], BF16, tag=f"pre{q}",
                                          name=f"pre{q}"))
                    nc.scalar.copy(out=pre[q][:], in_=pw[:])
                if 0 <= q - 1 < nW2:
                    stage_w1(q - 1)
                if 0 <= q - 2 < nW2:
                    stage_w2(q - 2)
                if 0 <= q - 3 < nW2:
                    stage_a2(q - 3)

            out_sb = const.tile([R_PAD, OUT_C], F32, tag="outsb")
            nc.vector.tensor_add(out=out_sb[:], in0=out_ps[:], in1=b2_sb)
            nc.sync.dma_start(out_d, out_sb[:])

    nc.compile()
    return nc


# ----------------------------------------------------------------------------
# Entry point
# ----------------------------------------------------------------------------

_RESULT_CACHE = {}


def kernel(x, edge_index, batch, num_graphs, W1, b1, W2, b2, **_ignored):
    x = np.ascontiguousarray(np.asarray(x, dtype=np.float32))
    edge_index = np.asarray(edge_index).astype(np.int64)
    batch = np.asarray(batch).astype(np.int64)
    G = int(np.asarray(num_graphs))
    W1 = np.asarray(W1, dtype=np.float32)
    b1 = np.asarray(b1, dtype=np.float32)
    W2 = np.asarray(W2, dtype=np.float32)
    b2 = np.asarray(b2, dtype=np.float32)

    per_core, meta = _build_shards(x, edge_index, batch, G, W1, W2, b1, b2)
    nc = _build_program(meta["nW"], meta["nW2"], meta["B"], meta["n_blk"])

    in_maps = [dict(pc) for pc in per_core]

    _ensure_ntff_hook()
    try:
        res = bass_utils.run_bass_kernel_spmd(nc, in_maps,
                                              core_ids=list(range(N_CORES)))
    except Exception:
        # transient device wedge (NRT_EXEC_UNIT_UNRECOVERABLE) or profiling
        # hiccup: retry once with tracing off and a core reset requested
        import os as _os
        _os.environ["BASS_NEVER_TRACE"] = "1"
        _os.environ.setdefault("NEURON_RT_RESET_CORES", "1")
        res = bass_utils.run_bass_kernel_spmd(nc, in_maps,
                                              core_ids=list(range(N_CORES)))
    # reassemble: core c's output row i corresponds to unique root
    # meta["root_of"][c][i]; uroots (sorted) is indexed by inv_map
    out_u = np.zeros((meta["U"], OUT_C), dtype=np.float32)
    uroots = np.sort(np.concatenate(meta["root_of"]))
    pos_of = {int(r): j for j, r in enumerate(uroots)}
    for c in range(N_CORES):
        o = np.asarray(res.results[c]["out"])
        for i, r in enumerate(meta["root_of"][c]):
            out_u[pos_of[int(r)]] = o[i]
    out = out_u[meta["inv_map"]].astype(np.float32)
    # kernel() may be probed; stash the bass results for test harness use
    _RESULT_CACHE["last"] = res
    return out


# revision 7
# speedup vs baseline: 1.2144x; 1.0713x over previous
"""Trainium2 Bass kernel for a 2-layer GCN with root-node readout.

The reference computes a full-graph 2-layer GCN but only returns h2[roots]
(one root per graph).  Exact algebraic pruning: out[g] depends only on edges
into root g (layer 2) and edges into those edges' sources (layer 1):

  out[g]  = sum_{e2: dst=root_g} norm_e2 * h2[src_e2] + b2
  h2      = relu( (sum_{e1: dst=s} norm_e1 * x[src_e1]) @ W1 + b1 ) @ W2

Sharding: unique roots are split across 8 cores, balanced by layer-1 edge
count.  Each core streams its layer-1 messages (norm*x rows) as 128-edge
blocks grouped into 64-destination windows.  Messages AND their one-hot
destination-selection matrices ship as fp8e4 (the 2e-2 harness tolerance
leaves ~2x margin over the ~1.1e-2 fp8 error), interleaved per block so one
DMA per window-pair feeds both matmul operands.  Scatter-add is a one-hot
matmul per block into a per-pair PSUM tile; fp8 weights get 4x fast-weight-
load so the PE tracks the DMA stream.  Layer 2 is folded into a small dense
matrix A2 [roots x sources] built on the host from edge norms; the
W1/relu/W2/A2 chain runs as a 4-deep software pipeline interleaved with the
scatter stream.
"""

import numpy as np
import ml_dtypes

import concourse.bacc as bacc
import concourse.bass as bass  # noqa: F401
import concourse.mybir as mybir
import concourse.tile as tile
from concourse import bass_utils
from concourse._compat import axon_active


def _ensure_ntff_hook():
    """bass_utils' trace path imports antenv.axon_hooks, which this image
    lacks; synthesize it from trn_agent_boot's ctypes recipe so BASS_TRACE=1
    profiling works. Silent no-op when anything is missing."""
    import sys as _sys
    try:
        import antenv.axon_hooks  # noqa: F401
        return
    except ImportError:
        pass
    try:
        import types as _types
        from trn_agent_boot.trn_boot import _ntff_profile_via_ctypes
        _hook = _ntff_profile_via_ctypes("/opt/axon/libaxon_pjrt.so")
        mod = _types.ModuleType("antenv.axon_hooks")
        mod.get_axon_ntff_profile_hook = lambda: _hook
        mod.set_axon_ntff_profile_hook = lambda h: None
        _sys.modules["antenv.axon_hooks"] = mod
        import antenv as _antenv
        _antenv.axon_hooks = mod
    except Exception:
        pass

N_CORES = 8
P = 128
W64 = 64
HID = 128
OUT_C = 64
R_PAD = 64
BLK_W = HID + W64          # per-block row: [msg 128 | one-hot 64] fp8

F32 = mybir.dt.float32
BF16 = mybir.dt.bfloat16
FP8 = mybir.dt.float8e4


# ----------------------------------------------------------------------------
# Host-side preprocessing
# ----------------------------------------------------------------------------

def _compute_norm_and_roots(x, edge_index, batch, num_graphs):
    """Replicate reference._gcn_norm and the root-finding logic exactly."""
    n = x.shape[0]
    G = int(num_graphs)
    loop = np.arange(n, dtype=np.int64)
    src = np.concatenate([edge_index[0], loop])
    dst = np.concatenate([edge_index[1], loop])
    deg = np.bincount(dst, minlength=n).astype(np.float64)
    dinv = np.zeros(n, dtype=np.float32)
    nz = deg > 0
    dinv[nz] = (1.0 / np.sqrt(deg[nz])).astype(np.float32)
    norm = (dinv[src] * dinv[dst]).astype(np.float32)

    node_types = x[:, 0]
    idx = np.arange(n, dtype=np.int64)
    cand = np.where(node_types == 0.0, idx, n)
    roots = np.full(G, np.iinfo(np.int64).max, dtype=np.int64)
    bc = np.clip(batch, 0, G - 1)
    np.minimum.at(roots, bc, cand)
    valid = np.zeros(G, dtype=bool)
    valid[bc] = True
    roots[~valid] = np.iinfo(np.int64).max
    roots = np.clip(roots, 0, n - 1)  # jax out-of-bounds gather clamps
    return src, dst, norm, roots, deg.astype(np.int64)


def _build_shards(x, edge_index, batch, num_graphs, W1, W2, b1, b2):
    n = x.shape[0]
    src, dst, norm, roots, deg = _compute_norm_and_roots(
        x, edge_index, batch, num_graphs)

    uroots, inv_map = np.unique(roots, return_inverse=True)
    U = len(uroots)

    order = np.argsort(dst, kind="stable")
    dst_s = dst[order]
    src_s = src[order]
    norm_s = norm[order]
    starts = np.searchsorted(dst_s, np.arange(n))
    ends = np.searchsorted(dst_s, np.arange(n) + 1)

    # balance roots across cores by estimated layer-1 edge load
    root_load = np.array(
        [deg[src_s[starts[r]:ends[r]]].sum() for r in uroots], dtype=np.int64)
    core_of_root = np.zeros(U, dtype=np.int64)
    core_load = np.zeros(N_CORES, dtype=np.int64)
    core_cnt = np.zeros(N_CORES, dtype=np.int64)
    for ri in np.argsort(-root_load):
        ok = core_cnt < R_PAD
        c = np.flatnonzero(ok)[np.argmin(core_load[ok])]
        core_of_root[ri] = c
        core_load[c] += root_load[ri]
        core_cnt[c] += 1

    cores = []
    for c in range(N_CORES):
        R_c = uroots[core_of_root == c]
        if len(R_c):
            e2_idx = np.concatenate(
                [np.arange(starts[r], ends[r]) for r in R_c])
        else:
            e2_idx = np.array([], dtype=np.int64)
        e2_src = src_s[e2_idx]
        e2_dst = dst_s[e2_idx]
        e2_norm = norm_s[e2_idx]
        S = np.unique(e2_src)
        cores.append(dict(R_c=R_c, e2_src=e2_src, e2_dst=e2_dst,
                          e2_norm=e2_norm, S=S))

    nS_max = max(max(len(c["S"]) for c in cores), 1)
    nW2 = -(-nS_max // P)        # 128-wide pair chunks (phase 2 granularity)
    nW = 2 * nW2                 # 64-wide scatter windows

    # per-core window assignment: bin-pack S nodes into nW windows (<=64
    # nodes each) balancing per-window layer-1 edge counts
    for c in cores:
        S = c["S"]
        nS = len(S)
        w_nodes = np.zeros(nW, dtype=np.int64)
        w_edges = np.zeros(nW, dtype=np.int64)
        s_window = np.zeros(max(nS, 1), dtype=np.int64)
        s_slot = np.zeros(max(nS, 1), dtype=np.int64)
        degS = deg[S] if nS else np.zeros(0, dtype=np.int64)
        for si in np.argsort(-degS, kind="stable"):
            ok = w_nodes < W64
            w = np.flatnonzero(ok)[np.argmin(w_edges[ok])]
            s_window[si] = w
            s_slot[si] = w_nodes[w]
            w_nodes[w] += 1
            w_edges[w] += degS[si]
        c["s_pos"] = s_window * W64 + s_slot  # position of S[i] in [0, nW*64)
        c["w_edges"] = w_edges

        R_c = c["R_c"]
        A2 = np.zeros((R_PAD, nW * W64), dtype=np.float32)
        if nS:
            r_pos = np.searchsorted(R_c, c["e2_dst"])
            s_pos2 = c["s_pos"][np.searchsorted(S, c["e2_src"])]
            np.add.at(A2, (r_pos, s_pos2), c["e2_norm"])
        c["A2"] = A2

    B = np.zeros(nW, dtype=np.int64)
    for c in cores:
        B = np.maximum(B, -(-c["w_edges"] // P))
    B = np.maximum(B, 1)
    n_blk = int(B.sum())
    blk0 = np.concatenate([[0], np.cumsum(B)])  # first block of each window

    per_core = []
    for c in cores:
        S = c["S"]
        nS = len(S)
        msg = np.zeros((n_blk * P, HID), dtype=np.float32)
        onehot = np.zeros((n_blk * P, W64), dtype=ml_dtypes.float8_e4m3)
        if nS:
            e1_idx = np.concatenate(
                [np.arange(starts[s], ends[s]) for s in S])
            e1_src = src_s[e1_idx]
            e1_pos = c["s_pos"][np.searchsorted(S, dst_s[e1_idx])]
            e1_norm = norm_s[e1_idx]
            o = np.argsort(e1_pos // W64, kind="stable")
            e1_src, e1_pos, e1_norm = e1_src[o], e1_pos[o], e1_norm[o]
            w_of_e = e1_pos // W64
            for w in range(nW):
                sel = w_of_e == w
                k = int(sel.sum())
                if k:
                    base = blk0[w] * P
                    msg[base:base + k] = e1_norm[sel, None] * x[e1_src[sel]]
                    onehot[base + np.arange(k), e1_pos[sel] - w * W64] = 1.0
        ms = np.empty((P, n_blk, BLK_W), dtype=ml_dtypes.float8_e4m3)
        ms[:, :, :HID] = (msg.astype(ml_dtypes.float8_e4m3)
                          .reshape(n_blk, P, HID).transpose(1, 0, 2))
        ms[:, :, HID:] = onehot.reshape(n_blk, P, W64).transpose(1, 0, 2)
        A2T = (c["A2"].T.reshape(nW2, P, R_PAD)
               .transpose(1, 0, 2).astype(ml_dtypes.bfloat16))
        per_core.append(dict(ms=np.ascontiguousarray(ms), A2T=A2T,
                             R_c=c["R_c"]))

    # cbA (bf16): [W1 | A2T | W2]; cf32: [b1 | b2pad]
    W1b = W1.astype(ml_dtypes.bfloat16)
    W2b = W2.astype(ml_dtypes.bfloat16)
    b2pad = np.zeros((P, OUT_C), dtype=np.float32)
    b2pad[:R_PAD] = np.tile(b2.astype(np.float32), (R_PAD, 1))
    cf32 = np.ascontiguousarray(np.concatenate(
        [b1.astype(np.float32).reshape(HID, 1), b2pad], axis=1))
    for pc in per_core:
        pc["cbA"] = np.ascontiguousarray(np.concatenate(
            [W1b, pc.pop("A2T").reshape(P, nW2 * R_PAD), W2b], axis=1))
        pc["cf32"] = cf32
    meta = dict(nW=nW, nW2=nW2, B=[int(v) for v in B], n_blk=n_blk, U=U,
                inv_map=inv_map,
                root_of=[pc.pop("R_c") for pc in per_core])
    return per_core, meta


# ----------------------------------------------------------------------------
# Device program
# ----------------------------------------------------------------------------

def _build_program(nW, nW2, B, n_blk):
    nc = bacc.Bacc("TRN2", target_bir_lowering=False, debug=not axon_active(),
                   num_devices=N_CORES)
    ms_d = nc.dram_tensor("ms", [P, n_blk, BLK_W], FP8,
                          kind="ExternalInput").ap()
    cba_w = HID + nW2 * R_PAD + OUT_C
    cba_d = nc.dram_tensor("cbA", [P, cba_w], BF16, kind="ExternalInput").ap()
    cf32_d = nc.dram_tensor("cf32", [P, 1 + OUT_C], F32,
                            kind="ExternalInput").ap()
    out_d = nc.dram_tensor("out", [R_PAD, OUT_C], F32, kind="ExternalOutput").ap()

    blk0 = [0]
    for w in range(nW):
        blk0.append(blk0[-1] + int(B[w]))

    with tile.TileContext(nc) as tc:
        with (
            tc.tile_pool(name="const", bufs=1) as const,
            tc.tile_pool(name="small", bufs=3) as small,
            tc.tile_pool(name="psagg", bufs=3, space="PSUM") as psagg,
            tc.tile_pool(name="ps1", bufs=2, space="PSUM") as ps1,
            tc.tile_pool(name="psout", bufs=1, space="PSUM") as psout,
        ):
            # constants ride the (otherwise idle) GPSIMD SWDGE ring so they
            # transfer concurrently with the first msg chunk on the sync ring
            cba = const.tile([P, cba_w], BF16, tag="cbA")
            nc.gpsimd.dma_start(cba[:], cba_d)
            w1_sb = cba[:, 0:HID]
            a2_sb = cba[:, HID:HID + nW2 * R_PAD]
            w2_sb = cba[:, HID + nW2 * R_PAD:]
            cf32 = const.tile([P, 1 + OUT_C], F32, tag="cf32")
            nc.gpsimd.dma_start(cf32[:], cf32_d)
            b1_sb = cf32[:, 0:1]
            b2_sb = cf32[:R_PAD, 1:1 + OUT_C]

            # msg+onehot DMA in window-pair chunks (~0.44 MB each), spread
            # across the sync HWDGE ring and the gpsimd SWDGE ring so both
            # drain concurrently.  Pair 0 is further split per-window so the
            # first scatter matmul's completion semaphore fires earlier.
            ms_p = []
            for p in range(nW2):
                b0, b1e = blk0[2 * p], blk0[2 * p + 2]
                mt = const.tile([P, b1e - b0, BLK_W], FP8, tag=f"ms{p}",
                                name=f"ms{p}")
                eng = nc.sync if p % 2 == 0 else nc.gpsimd
                if p == 0:
                    bm = blk0[1] - b0
                    nc.sync.dma_start(mt[:, :bm, :], ms_d[:, b0:blk0[1], :])
                    nc.sync.dma_start(mt[:, bm:, :], ms_d[:, blk0[1]:b1e, :])
                else:
                    eng.dma_start(mt[:], ms_d[:, b0:b1e, :])
                ms_p.append(mt)

            out_ps = psout.tile([R_PAD, OUT_C], F32, tag="outps")

            # Phase-2 (W1/relu/W2/A2) is a 4-deep software pipeline: its
            # stages for pairs q-1/q-2/q-3 are emitted between pair q's
            # scatter blocks so every cross-engine dependency has a full
            # pair-step of slack and the in-order PE queue never stalls.
            pre = []
            relu_t = {}
            h2sb_t = {}

            def stage_w1(q):
                p_agg1 = ps1.tile([HID, P], F32, tag="agg1", name=f"agg1_{q}")
                nc.tensor.matmul(out=p_agg1[:], lhsT=w1_sb, rhs=pre[q][:],
                                 start=True, stop=True)
                relu_w = small.tile([HID, P], BF16, tag="relu",
                                    name=f"relu_{q}")
                nc.scalar.activation(out=relu_w[:], in_=p_agg1[:],
                                     func=mybir.ActivationFunctionType.Relu,
                                     bias=b1_sb, scale=1.0)
                relu_t[q] = relu_w

            def stage_w2(q):
                p_h2 = ps1.tile([P, OUT_C], F32, tag="h2", name=f"h2_{q}")
                nc.tensor.matmul(out=p_h2[:], lhsT=relu_t.pop(q)[:],
                                 rhs=w2_sb, start=True, stop=True)
                h2_sb = small.tile([P, OUT_C], BF16, tag="h2sb",
                                   name=f"h2sb_{q}")
                nc.scalar.copy(out=h2_sb[:], in_=p_h2[:])
                h2sb_t[q] = h2_sb

            def stage_a2(q):
                nc.tensor.matmul(out=out_ps[:],
                                 lhsT=a2_sb[:, q * R_PAD:(q + 1) * R_PAD],
                                 rhs=h2sb_t.pop(q)[:],
                                 start=(q == 0), stop=(q == nW2 - 1))

            for q in range(nW2 + 3):
                if q < nW2:
                    mt = ms_p[q]
                    pw = psagg.tile([P, P], F32, tag="pw", name=f"pw{q}")
                    for h in range(2):
                        w = 2 * q + h
                        Bw = int(B[w])
                        mb0 = blk0[w] - blk0[2 * q]
                        cols = slice(h * W64, (h + 1) * W64)
                        for b in range(Bw):
                            nc.tensor.matmul(out=pw[:, cols],
                                             lhsT=mt[:, mb0 + b, 0:HID],
                                             rhs=mt[:, mb0 + b, HID:BLK_W],
                                             start=(b == 0),
                                             stop=(b == Bw - 1))
                    pre.append(const.tile([P, P], BF16, tag=f"pre{q}",
                                          name=f"pre{q}"))
                    nc.scalar.copy(out=pre[q][:], in_=pw[:])
                if 0 <= q - 1 < nW2:
                    stage_w1(q - 1)
                if 0 <= q - 2 < nW2:
                    stage_w2(q - 2)
                if 0 <= q - 3 < nW2:
                    stage_a2(q - 3)

            out_sb = const.tile([R_PAD, OUT_C], F32, tag="outsb")
            nc.vector.tensor_add(out=out_sb[:], in0=out_ps[:], in1=b2_sb)
            nc.sync.dma_start(out_d, out_sb[:])

    nc.compile()
    return nc


# ----------------------------------------------------------------------------
# Entry point
# ----------------------------------------------------------------------------

_RESULT_CACHE = {}


def kernel(x, edge_index, batch, num_graphs, W1, b1, W2, b2, **_ignored):
    x = np.ascontiguousarray(np.asarray(x, dtype=np.float32))
    edge_index = np.asarray(edge_index).astype(np.int64)
    batch = np.asarray(batch).astype(np.int64)
    G = int(np.asarray(num_graphs))
    W1 = np.asarray(W1, dtype=np.float32)
    b1 = np.asarray(b1, dtype=np.float32)
    W2 = np.asarray(W2, dtype=np.float32)
    b2 = np.asarray(b2, dtype=np.float32)

    per_core, meta = _build_shards(x, edge_index, batch, G, W1, W2, b1, b2)
    nc = _build_program(meta["nW"], meta["nW2"], meta["B"], meta["n_blk"])

    in_maps = [dict(pc) for pc in per_core]

    _ensure_ntff_hook()
    try:
        res = bass_utils.run_bass_kernel_spmd(nc, in_maps,
                                              core_ids=list(range(N_CORES)))
    except Exception:
        # transient device wedge (NRT_EXEC_UNIT_UNRECOVERABLE) or profiling
        # hiccup: retry once with tracing off and a core reset requested
        import os as _os
        _os.environ["BASS_NEVER_TRACE"] = "1"
        _os.environ.setdefault("NEURON_RT_RESET_CORES", "1")
        res = bass_utils.run_bass_kernel_spmd(nc, in_maps,
                                              core_ids=list(range(N_CORES)))
    # reassemble: core c's output row i corresponds to unique root
    # meta["root_of"][c][i]; uroots (sorted) is indexed by inv_map
    out_u = np.zeros((meta["U"], OUT_C), dtype=np.float32)
    uroots = np.sort(np.concatenate(meta["root_of"]))
    pos_of = {int(r): j for j, r in enumerate(uroots)}
    for c in range(N_CORES):
        o = np.asarray(res.results[c]["out"])
        for i, r in enumerate(meta["root_of"][c]):
            out_u[pos_of[int(r)]] = o[i]
    out = out_u[meta["inv_map"]].astype(np.float32)
    # kernel() may be probed; stash the bass results for test harness use
    _RESULT_CACHE["last"] = res
    return out


# revision 12
# speedup vs baseline: 1.2982x; 1.0691x over previous
"""Trainium2 Bass kernel for a 2-layer GCN with root-node readout.

The reference computes a full-graph 2-layer GCN but only returns h2[roots]
(one root per graph).  Exact algebraic pruning: out[g] depends only on edges
into root g (layer 2) and edges into those edges' sources (layer 1):

  out[g]  = sum_{e2: dst=root_g} norm_e2 * h2[src_e2] + b2
  h2      = relu( (sum_{e1: dst=s} norm_e1 * x[src_e1]) @ W1 + b1 ) @ W2

Sharding: unique roots are split across 8 cores, balanced by layer-1 edge
count.  Each core streams its layer-1 messages (norm*x rows) as 128-edge
blocks grouped into 64-destination windows.  Messages AND their one-hot
destination-selection matrices ship as fp8e4 (the 2e-2 harness tolerance
leaves ~2x margin over the ~1.1e-2 fp8 error), interleaved per block so one
DMA per window-pair feeds both matmul operands.  Scatter-add is a one-hot
matmul per block into a per-pair PSUM tile; fp8 weights get 4x fast-weight-
load so the PE tracks the DMA stream.  Layer 2 is folded into a small dense
matrix A2 [roots x sources] built on the host from edge norms; the
W1/relu/W2/A2 chain runs as a 4-deep software pipeline interleaved with the
scatter stream.
"""

import numpy as np
import ml_dtypes

import concourse.bacc as bacc
import concourse.bass as bass  # noqa: F401
import concourse.mybir as mybir
import concourse.tile as tile
from concourse import bass_utils
from concourse._compat import axon_active


def _ensure_ntff_hook():
    """bass_utils' trace path imports antenv.axon_hooks, which this image
    lacks; synthesize it from trn_agent_boot's ctypes recipe so BASS_TRACE=1
    profiling works. Silent no-op when anything is missing."""
    import sys as _sys
    try:
        import antenv.axon_hooks  # noqa: F401
        return
    except ImportError:
        pass
    try:
        import types as _types
        from trn_agent_boot.trn_boot import _ntff_profile_via_ctypes
        _hook = _ntff_profile_via_ctypes("/opt/axon/libaxon_pjrt.so")
        mod = _types.ModuleType("antenv.axon_hooks")
        mod.get_axon_ntff_profile_hook = lambda: _hook
        mod.set_axon_ntff_profile_hook = lambda h: None
        _sys.modules["antenv.axon_hooks"] = mod
        import antenv as _antenv
        _antenv.axon_hooks = mod
    except Exception:
        pass

N_CORES = 8
P = 128
W64 = 64
HID = 128
OUT_C = 64
R_PAD = 64
BLK_W = HID + W64          # per-block row: [msg 128 | one-hot 64] fp8

F32 = mybir.dt.float32
BF16 = mybir.dt.bfloat16
FP8 = mybir.dt.float8e4


# ----------------------------------------------------------------------------
# Host-side preprocessing
# ----------------------------------------------------------------------------

def _compute_norm_and_roots(x, edge_index, batch, num_graphs):
    """Replicate reference._gcn_norm and the root-finding logic exactly."""
    n = x.shape[0]
    G = int(num_graphs)
    loop = np.arange(n, dtype=np.int64)
    src = np.concatenate([edge_index[0], loop])
    dst = np.concatenate([edge_index[1], loop])
    deg = np.bincount(dst, minlength=n).astype(np.float64)
    dinv = np.zeros(n, dtype=np.float32)
    nz = deg > 0
    dinv[nz] = (1.0 / np.sqrt(deg[nz])).astype(np.float32)
    norm = (dinv[src] * dinv[dst]).astype(np.float32)

    node_types = x[:, 0]
    idx = np.arange(n, dtype=np.int64)
    cand = np.where(node_types == 0.0, idx, n)
    roots = np.full(G, np.iinfo(np.int64).max, dtype=np.int64)
    bc = np.clip(batch, 0, G - 1)
    np.minimum.at(roots, bc, cand)
    valid = np.zeros(G, dtype=bool)
    valid[bc] = True
    roots[~valid] = np.iinfo(np.int64).max
    roots = np.clip(roots, 0, n - 1)  # jax out-of-bounds gather clamps
    return src, dst, norm, roots, deg.astype(np.int64)


def _build_shards(x, edge_index, batch, num_graphs, W1, W2, b1, b2):
    n = x.shape[0]
    src, dst, norm, roots, deg = _compute_norm_and_roots(
        x, edge_index, batch, num_graphs)

    uroots, inv_map = np.unique(roots, return_inverse=True)
    U = len(uroots)

    order = np.argsort(dst, kind="stable")
    dst_s = dst[order]
    src_s = src[order]
    norm_s = norm[order]
    starts = np.searchsorted(dst_s, np.arange(n))
    ends = np.searchsorted(dst_s, np.arange(n) + 1)

    # balance roots across cores by estimated layer-1 edge load
    root_load = np.array(
        [deg[src_s[starts[r]:ends[r]]].sum() for r in uroots], dtype=np.int64)
    core_of_root = np.zeros(U, dtype=np.int64)
    core_load = np.zeros(N_CORES, dtype=np.int64)
    core_cnt = np.zeros(N_CORES, dtype=np.int64)
    for ri in np.argsort(-root_load):
        ok = core_cnt < R_PAD
        c = np.flatnonzero(ok)[np.argmin(core_load[ok])]
        core_of_root[ri] = c
        core_load[c] += root_load[ri]
        core_cnt[c] += 1

    cores = []
    for c in range(N_CORES):
        R_c = uroots[core_of_root == c]
        if len(R_c):
            e2_idx = np.concatenate(
                [np.arange(starts[r], ends[r]) for r in R_c])
        else:
            e2_idx = np.array([], dtype=np.int64)
        e2_src = src_s[e2_idx]
        e2_dst = dst_s[e2_idx]
        e2_norm = norm_s[e2_idx]
        S = np.unique(e2_src)
        cores.append(dict(R_c=R_c, e2_src=e2_src, e2_dst=e2_dst,
                          e2_norm=e2_norm, S=S))

    nS_max = max(max(len(c["S"]) for c in cores), 1)
    nW2 = -(-nS_max // P)        # 128-wide pair chunks (phase 2 granularity)
    nW = 2 * nW2                 # 64-wide scatter windows

    # per-core window assignment: bin-pack S nodes into nW windows (<=64
    # nodes each) balancing per-window layer-1 edge counts
    for c in cores:
        S = c["S"]
        nS = len(S)
        w_nodes = np.zeros(nW, dtype=np.int64)
        w_edges = np.zeros(nW, dtype=np.int64)
        s_window = np.zeros(max(nS, 1), dtype=np.int64)
        s_slot = np.zeros(max(nS, 1), dtype=np.int64)
        degS = deg[S] if nS else np.zeros(0, dtype=np.int64)
        for si in np.argsort(-degS, kind="stable"):
            ok = w_nodes < W64
            w = np.flatnonzero(ok)[np.argmin(w_edges[ok])]
            s_window[si] = w
            s_slot[si] = w_nodes[w]
            w_nodes[w] += 1
            w_edges[w] += degS[si]
        c["s_pos"] = s_window * W64 + s_slot  # position of S[i] in [0, nW*64)
        c["w_edges"] = w_edges

        R_c = c["R_c"]
        A2 = np.zeros((R_PAD, nW * W64), dtype=np.float32)
        if nS:
            r_pos = np.searchsorted(R_c, c["e2_dst"])
            s_pos2 = c["s_pos"][np.searchsorted(S, c["e2_src"])]
            np.add.at(A2, (r_pos, s_pos2), c["e2_norm"])
        c["A2"] = A2

    B = np.zeros(nW, dtype=np.int64)
    for c in cores:
        B = np.maximum(B, -(-c["w_edges"] // P))
    B = np.maximum(B, 1)
    n_blk = int(B.sum())
    blk0 = np.concatenate([[0], np.cumsum(B)])  # first block of each window

    per_core = []
    for c in cores:
        S = c["S"]
        nS = len(S)
        msg = np.zeros((n_blk * P, HID), dtype=np.float32)
        dstrel = np.zeros(n_blk * P, dtype=np.float32)
        if nS:
            e1_idx = np.concatenate(
                [np.arange(starts[s], ends[s]) for s in S])
            e1_src = src_s[e1_idx]
            e1_pos = c["s_pos"][np.searchsorted(S, dst_s[e1_idx])]
            e1_norm = norm_s[e1_idx]
            o = np.argsort(e1_pos // W64, kind="stable")
            e1_src, e1_pos, e1_norm = e1_src[o], e1_pos[o], e1_norm[o]
            w_of_e = e1_pos // W64
            for w in range(nW):
                sel = w_of_e == w
                k = int(sel.sum())
                if k:
                    base = blk0[w] * P
                    msg[base:base + k] = e1_norm[sel, None] * x[e1_src[sel]]
                    dstrel[base:base + k] = (e1_pos[sel] - w * W64).astype(
                        np.float32)
        ms = np.ascontiguousarray(
            msg.astype(ml_dtypes.float8_e4m3)
            .reshape(n_blk, P, HID).transpose(1, 0, 2))
        dr = np.ascontiguousarray(
            dstrel.reshape(n_blk, P).T.astype(ml_dtypes.bfloat16))
        A2T = (c["A2"].T.reshape(nW2, P, R_PAD)
               .transpose(1, 0, 2).astype(ml_dtypes.bfloat16))
        per_core.append(dict(ms=ms, dr=dr, A2T=A2T, R_c=c["R_c"]))

    # cbA (bf16): [dr | iota | W1 | A2T | W2]; cf32: [b1 | b2pad]
    iota = np.tile(np.arange(W64, dtype=np.float32), (P, 1)).astype(
        ml_dtypes.bfloat16)
    W1b = W1.astype(ml_dtypes.bfloat16)
    W2b = W2.astype(ml_dtypes.bfloat16)
    b2pad = np.zeros((P, OUT_C), dtype=np.float32)
    b2pad[:R_PAD] = np.tile(b2.astype(np.float32), (R_PAD, 1))
    cf32 = np.ascontiguousarray(np.concatenate(
        [b1.astype(np.float32).reshape(HID, 1), b2pad], axis=1))
    for pc in per_core:
        pc["cbA"] = np.ascontiguousarray(np.concatenate(
            [pc.pop("dr"), iota, W1b,
             pc.pop("A2T").reshape(P, nW2 * R_PAD), W2b], axis=1))
        pc["cf32"] = cf32
    meta = dict(nW=nW, nW2=nW2, B=[int(v) for v in B], n_blk=n_blk, U=U,
                inv_map=inv_map,
                root_of=[pc.pop("R_c") for pc in per_core])
    return per_core, meta


# ----------------------------------------------------------------------------
# Device program
# ----------------------------------------------------------------------------

def _build_program(nW, nW2, B, n_blk):
    nc = bacc.Bacc("TRN2", target_bir_lowering=False, debug=not axon_active(),
                   num_devices=N_CORES)
    ms_d = nc.dram_tensor("ms", [P, n_blk, HID], FP8,
                          kind="ExternalInput").ap()
    cba_w = n_blk + W64 + HID + nW2 * R_PAD + OUT_C
    cba_d = nc.dram_tensor("cbA", [P, cba_w], BF16, kind="ExternalInput").ap()
    cf32_d = nc.dram_tensor("cf32", [P, 1 + OUT_C], F32,
                            kind="ExternalInput").ap()
    out_d = nc.dram_tensor("out", [R_PAD, OUT_C], F32, kind="ExternalOutput").ap()

    blk0 = [0]
    for w in range(nW):
        blk0.append(blk0[-1] + int(B[w]))

    with tile.TileContext(nc) as tc:
        with (
            tc.tile_pool(name="const", bufs=1) as const,
            tc.tile_pool(name="small", bufs=3) as small,
            tc.tile_pool(name="psagg", bufs=3, space="PSUM") as psagg,
            tc.tile_pool(name="ps1", bufs=2, space="PSUM") as ps1,
            tc.tile_pool(name="psout", bufs=1, space="PSUM") as psout,
        ):
            # the S-gen-critical blob ships FIRST on the fast sync ring; the
            # fp32 biases ride the (otherwise idle) GPSIMD SWDGE ring
            cba = const.tile([P, cba_w], BF16, tag="cbA")
            nc.sync.dma_start(cba[:], cba_d)
            dr_sb = cba[:, 0:n_blk]
            o = n_blk
            iota_sb = cba[:, o:o + W64]; o += W64
            w1_sb = cba[:, o:o + HID]; o += HID
            a2_sb = cba[:, o:o + nW2 * R_PAD]; o += nW2 * R_PAD
            w2_sb = cba[:, o:o + OUT_C]
            cf32 = const.tile([P, 1 + OUT_C], F32, tag="cf32")
            nc.gpsimd.dma_start(cf32[:], cf32_d)
            b1_sb = cf32[:, 0:1]
            b2_sb = cf32[:R_PAD, 1:1 + OUT_C]

            # msg DMA in window-pair chunks on the sync ring.  Pair 0 is
            # split per-window so the first scatter's semaphore fires early.
            ms_p = []
            for p in range(nW2):
                b0, b1e = blk0[2 * p], blk0[2 * p + 2]
                mt = const.tile([P, b1e - b0, HID], FP8, tag=f"ms{p}",
                                name=f"ms{p}")
                if p == 0:
                    bm = blk0[1] - b0
                    nc.sync.dma_start(mt[:, :bm, :], ms_d[:, b0:blk0[1], :])
                    nc.sync.dma_start(mt[:, bm:, :], ms_d[:, blk0[1]:b1e, :])
                else:
                    nc.sync.dma_start(mt[:], ms_d[:, b0:b1e, :])
                ms_p.append(mt)

            # S generation on the DVE: one-hot(dstrel) in fp8, one batched
            # is_equal per pair, overlapped with the msg DMA stream
            s_p = []
            for p in range(nW2):
                b0, b1e = blk0[2 * p], blk0[2 * p + 2]
                st = const.tile([P, b1e - b0, W64], FP8, tag=f"S{p}",
                                name=f"S{p}")
                nc.vector.tensor_tensor(
                    out=st[:],
                    in0=dr_sb[:, b0:b1e, None].to_broadcast(
                        [P, b1e - b0, W64]),
                    in1=iota_sb[:, None, :].to_broadcast([P, b1e - b0, W64]),
                    op=mybir.AluOpType.is_equal)
                s_p.append(st)

            out_ps = psout.tile([R_PAD, OUT_C], F32, tag="outps")

            # Phase-2 (W1/relu/W2/A2) is a 4-deep software pipeline: its
            # stages for pairs q-1/q-2/q-3 are emitted between pair q's
            # scatter blocks so every cross-engine dependency has a full
            # pair-step of slack and the in-order PE queue never stalls.
            pre = []
            relu_t = {}
            h2sb_t = {}

            def stage_w1(q):
                p_agg1 = ps1.tile([HID, P], F32, tag="agg1", name=f"agg1_{q}")
                nc.tensor.matmul(out=p_agg1[:], lhsT=w1_sb, rhs=pre[q][:],
                                 start=True, stop=True)
                relu_w = small.tile([HID, P], BF16, tag="relu",
                                    name=f"relu_{q}")
                nc.scalar.activation(out=relu_w[:], in_=p_agg1[:],
                                     func=mybir.ActivationFunctionType.Relu,
                                     bias=b1_sb, scale=1.0)
                relu_t[q] = relu_w

            def stage_w2(q):
                p_h2 = ps1.tile([P, OUT_C], F32, tag="h2", name=f"h2_{q}")
                nc.tensor.matmul(out=p_h2[:], lhsT=relu_t.pop(q)[:],
                                 rhs=w2_sb, start=True, stop=True)
                h2_sb = small.tile([P, OUT_C], BF16, tag="h2sb",
                                   name=f"h2sb_{q}")
                nc.scalar.copy(out=h2_sb[:], in_=p_h2[:])
                h2sb_t[q] = h2_sb

            def stage_a2(q):
                nc.tensor.matmul(out=out_ps[:],
                                 lhsT=a2_sb[:, q * R_PAD:(q + 1) * R_PAD],
                                 rhs=h2sb_t.pop(q)[:],
                                 start=(q == 0), stop=(q == nW2 - 1))

            for q in range(nW2 + 3):
                if q < nW2:
                    mt = ms_p[q]
                    pw = psagg.tile([P, P], F32, tag="pw", name=f"pw{q}")
                    for h in range(2):
                        w = 2 * q + h
                        Bw = int(B[w])
                        mb0 = blk0[w] - blk0[2 * q]
                        cols = slice(h * W64, (h + 1) * W64)
                        for b in range(Bw):
                            nc.tensor.matmul(out=pw[:, cols],
                                             lhsT=mt[:, mb0 + b, :],
                                             rhs=s_p[q][:, mb0 + b, :],
                                             start=(b == 0),
                                             stop=(b == Bw - 1))
                    pre.append(const.tile([P, P], BF16, tag=f"pre{q}",
                                          name=f"pre{q}"))
                    nc.scalar.copy(out=pre[q][:], in_=pw[:])
                if 0 <= q - 1 < nW2:
                    stage_w1(q - 1)
                if 0 <= q - 2 < nW2:
                    stage_w2(q - 2)
                if 0 <= q - 3 < nW2:
                    stage_a2(q - 3)

            out_sb = const.tile([R_PAD, OUT_C], F32, tag="outsb")
            nc.vector.tensor_add(out=out_sb[:], in0=out_ps[:], in1=b2_sb)
            nc.sync.dma_start(out_d, out_sb[:])

    nc.compile()
    return nc


# ----------------------------------------------------------------------------
# Entry point
# ----------------------------------------------------------------------------

_RESULT_CACHE = {}


def kernel(x, edge_index, batch, num_graphs, W1, b1, W2, b2, **_ignored):
    x = np.ascontiguousarray(np.asarray(x, dtype=np.float32))
    edge_index = np.asarray(edge_index).astype(np.int64)
    batch = np.asarray(batch).astype(np.int64)
    G = int(np.asarray(num_graphs))
    W1 = np.asarray(W1, dtype=np.float32)
    b1 = np.asarray(b1, dtype=np.float32)
    W2 = np.asarray(W2, dtype=np.float32)
    b2 = np.asarray(b2, dtype=np.float32)

    per_core, meta = _build_shards(x, edge_index, batch, G, W1, W2, b1, b2)
    nc = _build_program(meta["nW"], meta["nW2"], meta["B"], meta["n_blk"])

    in_maps = [dict(pc) for pc in per_core]

    _ensure_ntff_hook()
    try:
        res = bass_utils.run_bass_kernel_spmd(nc, in_maps,
                                              core_ids=list(range(N_CORES)))
    except Exception:
        # transient device wedge (NRT_EXEC_UNIT_UNRECOVERABLE) or profiling
        # hiccup: retry once with tracing off and a core reset requested
        import os as _os
        _os.environ["BASS_NEVER_TRACE"] = "1"
        _os.environ.setdefault("NEURON_RT_RESET_CORES", "1")
        res = bass_utils.run_bass_kernel_spmd(nc, in_maps,
                                              core_ids=list(range(N_CORES)))
    # reassemble: core c's output row i corresponds to unique root
    # meta["root_of"][c][i]; uroots (sorted) is indexed by inv_map
    out_u = np.zeros((meta["U"], OUT_C), dtype=np.float32)
    uroots = np.sort(np.concatenate(meta["root_of"]))
    pos_of = {int(r): j for j, r in enumerate(uroots)}
    for c in range(N_CORES):
        o = np.asarray(res.results[c]["out"])
        for i, r in enumerate(meta["root_of"][c]):
            out_u[pos_of[int(r)]] = o[i]
    out = out_u[meta["inv_map"]].astype(np.float32)
    # kernel() may be probed; stash the bass results for test harness use
    _RESULT_CACHE["last"] = res
    return out
